# revision 14
# baseline (speedup 1.0000x reference)
import sys

sys.path.insert(0, "/opt/trn_rl_repo")

import numpy as np  # noqa: E402
import ml_dtypes  # noqa: E402

import concourse.mybir as mybir  # noqa: E402
import concourse.tile as tile  # noqa: E402
from contextlib import ExitStack  # noqa: E402
from concourse import bacc  # noqa: E402
from concourse.bass_utils import run_bass_kernel_spmd  # noqa: E402
from concourse.masks import make_identity  # noqa: E402

F32 = mybir.dt.float32
BF16 = mybir.dt.bfloat16
F8 = mybir.dt.float8e4
AF = mybir.ActivationFunctionType
ALU = mybir.AluOpType
AX = mybir.AxisListType

S = 4  # samples per core
C, H, W = 256, 28, 28
N = H * W  # 784
NK = 196
HEADS, DK = 8, 32
CM = 1024
SCALE = DK ** -0.5
EPS = 1e-5
INV_NTOT = 1.0 / (C * N)
ISL = [(0, 512), (512, 272)]  # bank-aligned free splits of 784
NCORES = 8

# ---- mega-constant layouts (bf16 columns) ----
# on-device diag tiles (built from compact dgw columns)
O_DGLPU = 0           # 2 groups x 9 taps x 128
O_DGKV = 2304         # 2 groups x 4 taps x 128
CEA_COLS = 3328
DGW_COLS = 98         # lpu 2x9, kv 2x4, dw2 8x9 (f32 weight columns)
# EARLY-B tile
O_WQT = 0             # 2 x 256
O_WKT = 512
O_WVT = 1024
O_BH = 1536           # 4 x 128
O_ROWB = 2048         # row 0: bo(256), bv(256)
CEB_COLS = 2560
# MID tile
O_EC = 0              # 8 x 1568
O_WOT = 12544         # 2 x 256
CMID_COLS = 13056
# LATE tile (c1t + c2t only; dw2 diagonals built on device)
O_C1T = 0             # 2 x 1024
O_C2T = 2048          # 8 x 256
CL_COLS = 4096
DGW2_COLS = 9216      # on-device dw2 diag tile

_CACHE = {}


def _build():
    if "nc" in _CACHE:
        return _CACHE["nc"]
    nc = bacc.Bacc()

    x_d = nc.dram_tensor("x", [S, C, H, W], F32, kind="ExternalInput")
    y_d = nc.dram_tensor("y", [S, C, H, W], F32, kind="ExternalOutput")
    dgw_d = nc.dram_tensor("dgw", [128, DGW_COLS], F32, kind="ExternalInput")
    ceb_d = nc.dram_tensor("ceb", [128, CEB_COLS], BF16, kind="ExternalInput")
    cm_d = nc.dram_tensor("cmid", [128, CMID_COLS], BF16, kind="ExternalInput")
    cl_d = nc.dram_tensor("clate", [128, CL_COLS], BF16, kind="ExternalInput")
    cb_d = nc.dram_tensor("cbias", [128, 10], F32, kind="ExternalInput")
    bn_d = nc.dram_tensor("bncol", [128, 44], F32, kind="ExternalInput")
    scr_d = nc.dram_tensor("scr", [S, N * C], F32)

    xv = x_d.rearrange("s c h w -> s c (h w)")
    yv = y_d.rearrange("s c h w -> s c (h w)")

    with tile.TileContext(nc) as tc, ExitStack() as stk:
        cst = stk.enter_context(tc.tile_pool(name="cst", bufs=1))
        wk = stk.enter_context(tc.tile_pool(name="wk", bufs=2))
        psA = stk.enter_context(tc.tile_pool(name="psA", bufs=3, space="PSUM"))
        psS = stk.enter_context(tc.tile_pool(name="psS", bufs=2, space="PSUM"))

        # sample-0 input load first so LPU can start ASAP
        def load_x(s):
            xs = []
            for ch in range(2):
                t = wk.tile([128, N], F32, tag=f"xf{ch}", name=f"xf{ch}")
                nc.sync.dma_start(
                    out=t, in_=xv[s, ch * 128:(ch + 1) * 128, :])
                xs.append(t)
            return xs

        x0 = load_x(0)

        dgw = cst.tile([128, DGW_COLS], F32, tag="dgw")
        cEa = cst.tile([128, CEA_COLS], BF16, tag="cEa")
        dgW2 = cst.tile([128, DGW2_COLS], BF16, tag="dgW2")
        cEb = cst.tile([128, CEB_COLS], BF16, tag="cEb")
        cMt = cst.tile([128, CMID_COLS], BF16, tag="cMt")
        cLt = cst.tile([128, CL_COLS], BF16, tag="cLt")
        cbias = cst.tile([128, 10], F32, tag="cbias")
        bncol = cst.tile([128, 44], F32, tag="bncol")
        nc.sync.dma_start(out=dgw, in_=dgw_d[:, :])
        nc.sync.dma_start(out=cbias, in_=cb_d[:, :])
        nc.sync.dma_start(out=bncol, in_=bn_d[:, :])
        nc.sync.dma_start(out=cEb, in_=ceb_d[:, :])
        HM = 6272  # first 4 heads of EC
        nc.scalar.dma_start(out=cMt[:, 0:HM], in_=cm_d[:, 0:HM])
        nc.scalar.dma_start(out=cMt[:, HM:], in_=cm_d[:, HM:])
        nc.scalar.dma_start(out=cLt, in_=cl_d[:, :])

        onesM = cst.tile([128, 128], F32, tag="onesM")
        nc.vector.memset(onesM, 1.0)
        ones1 = cst.tile([1, 128], BF16, tag="ones1")
        nc.vector.memset(ones1, 1.0)
        eps128 = cst.tile([128, 1], F32, tag="eps128")
        nc.vector.memset(eps128, EPS)
        ident = cst.tile([128, 128], BF16, tag="ident")
        make_identity(nc, ident)
        # build conv-tap diagonal matrices from compact weight columns
        for g in range(2):
            for t in range(9):
                o = O_DGLPU + g * 1152 + t * 128
                nc.vector.tensor_scalar(
                    out=cEa[:, o:o + 128], in0=ident,
                    scalar1=dgw[:, g * 9 + t:g * 9 + t + 1], scalar2=None,
                    op0=ALU.mult)
        for g in range(2):
            for t in range(4):
                o = O_DGKV + g * 512 + t * 128
                nc.vector.tensor_scalar(
                    out=cEa[:, o:o + 128], in0=ident,
                    scalar1=dgw[:, 18 + g * 4 + t:19 + g * 4 + t], scalar2=None,
                    op0=ALU.mult)
        for m in range(8):
            for t in range(9):
                o = m * 1152 + t * 128
                nc.gpsimd.tensor_scalar(
                    out=dgW2[:, o:o + 128], in0=ident,
                    scalar1=dgw[:, 26 + m * 9 + t:27 + m * 9 + t], scalar2=None,
                    op0=ALU.mult)

        # constant slice helpers
        def DGLPU(g, t):
            o = O_DGLPU + g * 1152 + t * 128
            return cEa[:, o:o + 128]

        def DGKV(g, t):
            o = O_DGKV + g * 512 + t * 128
            return cEa[:, o:o + 128]

        def WQT(kc):
            o = O_WQT + kc * 256
            return cEb[:, o:o + 256]

        def WKT(kc):
            o = O_WKT + kc * 256
            return cEb[:, o:o + 256]

        def WVT(kc):
            o = O_WVT + kc * 256
            return cEb[:, o:o + 256]

        def BH(q):
            o = O_BH + q * 128
            return cEb[:, o:o + 128]

        BO_R = cEb[0:1, O_ROWB:O_ROWB + 256]
        BV_R = cEb[0:1, O_ROWB + 256:O_ROWB + 512]

        def ECt(h):
            o = O_EC + h * 1568
            return cMt[:, o:o + 1568]

        def WOT(kc):
            o = O_WOT + kc * 256
            return cMt[:, o:o + 256]

        def C1T(kc):
            o = O_C1T + kc * 1024
            return cLt[:, o:o + 1024]

        def DGDW2(m, t):
            o = m * 1152 + t * 128
            return dgW2[:, o:o + 128]

        def C2T(kc):
            o = O_C2T + kc * 256
            return cLt[:, o:o + 256]

        def LPUB(g):
            return cbias[:, g:g + 1]

        def DWB(g):
            return cbias[:, 2 + g:3 + g]

        def BKC(g):
            return cbias[:, 4 + g:5 + g]

        RSWQN = cbias[:, 6:8]
        BQS = cbias[:, 8:10]

        A1cols = bncol[:, 0:8]
        B1cols = bncol[:, 8:16]

        def A2c(m):
            return bncol[:, 16 + m:17 + m]

        def B2c(m):
            return bncol[:, 24 + m:25 + m]

        def A3c(m):
            return bncol[:, 32 + m:33 + m]

        B3cols = bncol[:, 34:36]
        RSC1 = bncol[:, 36:44]

        def pat(name):
            return psA.tile([128, 1024], F32, tag="pat", name=name)

        def psm(name):
            return psS.tile([128, 512], F32, tag="psS", name=name)

        # partition-reduce [128, w] stats + final mean/var/rstd
        # stq cols: 0..3 partial sums, 4..5 partial sumsq
        def ln_finish(stq, tg):
            pst = psm("lnred")
            nc.tensor.matmul(pst[:, 0:6], onesM, stq[:, 0:6], start=True,
                             stop=True)
            sb = wk.tile([128, 8], F32, tag=f"lnsb{tg}")
            nc.vector.tensor_scalar(
                out=sb[:, 0:6], in0=pst[:, 0:6], scalar1=INV_NTOT, scalar2=None,
                op0=ALU.mult)
            nc.vector.tensor_add(out=sb[:, 6:8], in0=sb[:, 0:2], in1=sb[:, 2:4])
            mv = wk.tile([128, 4], F32, tag=f"lnmv{tg}")
            # mean, e2, var, rstd
            nc.vector.tensor_add(out=mv[:, 0:1], in0=sb[:, 6:7], in1=sb[:, 7:8])
            nc.vector.tensor_add(out=mv[:, 1:2], in0=sb[:, 4:5], in1=sb[:, 5:6])
            nc.vector.tensor_mul(out=mv[:, 2:3], in0=mv[:, 0:1], in1=mv[:, 0:1])
            nc.vector.tensor_sub(out=mv[:, 2:3], in0=mv[:, 1:2], in1=mv[:, 2:3])
            nc.scalar.activation(out=mv[:, 3:4], in_=mv[:, 2:3],
                                 func=AF.Abs_reciprocal_sqrt, bias=eps128)
            return mv[:, 0:1], mv[:, 3:4]

        # ---------------- per-sample stages ----------------
        def front(s, xs):
            st = {}
            if xs is None:
                xs = load_x(s)
            xb = []
            for ch in range(2):
                p = wk.tile([128, 30, 30], BF16, tag=f"xb{ch}")
                if s < 2:
                    nc.vector.memset(p, 0.0)
                nc.vector.tensor_copy(
                    out=p[:, 1:29, 1:29],
                    in_=xs[ch].rearrange("p (h w) -> p h w", w=W))
                xb.append(p)
            # LPU depthwise 3x3 + bias + residual -> x1 (bf16) with LN sums
            st6 = wk.tile([128, 8], F32, tag="st6a")
            x1 = []
            for ch in range(2):
                pl = pat("lpu")
                for t9 in range(9):
                    dy, dx = t9 // 3, t9 % 3
                    nc.tensor.matmul(
                        pl[:, 0:392], DGLPU(ch, t9),
                        xb[ch][:, dy:dy + 14, dx:dx + 28],
                        start=(t9 == 0), stop=(t9 == 8))
                    nc.tensor.matmul(
                        pl[:, 512:904], DGLPU(ch, t9),
                        xb[ch][:, dy + 14:dy + 28, dx:dx + 28],
                        start=(t9 == 0), stop=(t9 == 8))
                t = wk.tile([128, N], BF16, tag=f"x1{ch}")
                for hf in range(2):
                    sl = slice(hf * 392, (hf + 1) * 392)
                    c0 = hf * 512
                    nc.vector.scalar_tensor_tensor(
                        out=t[:, sl].rearrange("p (h w) -> p h w", w=W),
                        in0=pl[:, c0:c0 + 392].rearrange(
                            "p (h w) -> p h w", w=W),
                        scalar=LPUB(ch),
                        in1=xb[ch][:, 1 + 14 * hf:15 + 14 * hf, 1:29],
                        op0=ALU.add, op1=ALU.add,
                        accum_out=st6[:, 2 * ch + hf:2 * ch + hf + 1])
                x1.append(t)
            scr = wk.tile([128, N], BF16, tag="lnsc")
            for ch in range(2):
                nc.vector.scalar_tensor_tensor(
                    out=scr, in0=x1[ch], scalar=1.0, in1=x1[ch],
                    op0=ALU.mult, op1=ALU.mult,
                    accum_out=st6[:, 4 + ch:5 + ch])
            mean1, rst1 = ln_finish(st6, "l1")
            # fused q-proj LN coefficients
            mr = wk.tile([128, 2], F32, tag="qmr")
            nc.vector.tensor_mul(out=mr[:, 0:1], in0=mean1, in1=rst1)
            nc.vector.tensor_scalar(
                out=mr[:, 1:2], in0=rst1, scalar1=SCALE, scalar2=None,
                op0=ALU.mult)
            cq = wk.tile([128, 2], F32, tag="qcq")
            nc.vector.scalar_tensor_tensor(
                out=cq, in0=RSWQN, scalar=mr[:, 0:1], in1=BQS,
                op0=ALU.mult, op1=ALU.add)
            # kv conv (2x2 stride 2 on x1)
            kvb = []
            for ch in range(2):
                x5 = x1[ch].rearrange(
                    "p (h a w b) -> p h a w b", h=14, a=2, w=14, b=2)
                pk = psm("kv")
                for t4 in range(4):
                    nc.tensor.matmul(
                        pk[:, 0:NK], DGKV(ch, t4), x5[:, :, t4 // 2, :, t4 % 2],
                        start=(t4 == 0), stop=(t4 == 3))
                t = wk.tile([128, NK], BF16, tag=f"kvb{ch}")
                nc.vector.tensor_scalar(
                    out=t, in0=pk[:, 0:NK], scalar1=DWB(ch), scalar2=None,
                    op0=ALU.add)
                kvb.append(t)
            # q projection from x1 with fused LN affine
            qb = []
            for mc in range(2):
                pq = pat("q")
                for kc in range(2):
                    for i0, iw in ISL:
                        nc.tensor.matmul(
                            pq[:, i0:i0 + iw],
                            WQT(kc)[:, mc * 128:(mc + 1) * 128],
                            x1[kc][:, i0:i0 + iw],
                            start=(kc == 0), stop=(kc == 1))
                t = wk.tile([128, N], BF16, tag=f"qb{mc}")
                nc.vector.tensor_scalar(
                    out=t, in0=pq[:, 0:N], scalar1=mr[:, 1:2],
                    scalar2=cq[:, mc:mc + 1], op0=ALU.mult, op1=ALU.add)
                qb.append(t)
            kb = []
            for mc in range(2):
                pk2 = psm("k")
                for kc in range(2):
                    nc.tensor.matmul(
                        pk2[:, 0:NK], WKT(kc)[:, mc * 128:(mc + 1) * 128],
                        kvb[kc], start=(kc == 0), stop=(kc == 1))
                t = wk.tile([128, NK], BF16, tag=f"kb{mc}")
                nc.vector.tensor_scalar(
                    out=t, in0=pk2[:, 0:NK], scalar1=BKC(mc), scalar2=None,
                    op0=ALU.add)
                kb.append(t)
            vb = []
            for pi, (j0, jw) in enumerate([(0, 128), (128, 68)]):
                pv = psm("v")
                nc.tensor.matmul(
                    pv[0:jw, 0:C], ones1[0:1, 0:jw], BV_R, start=True,
                    stop=False)
                for kc in range(2):
                    nc.tensor.matmul(
                        pv[0:jw, 0:C], kvb[kc][:, j0:j0 + jw], WVT(kc),
                        start=False, stop=(kc == 1))
                t = wk.tile([128, C], BF16, tag=f"vb{pi}")
                nc.vector.tensor_copy(out=t[0:jw, :], in_=pv[0:jw, 0:C])
                vb.append(t)
            st["x1"], st["qb"], st["kb"], st["vb"] = x1, qb, kb, vb
            return st

        def attn(s, st):
            qb, kb, vb = st["qb"], st["kb"], st["vb"]
            pa = [None] * 8

            def f1_head(h):
                tc4, ro = h // 4, 32 * (h % 4)
                aA = pat("attA")
                aB = pat("attB")
                for i0, iw in ISL:
                    nc.tensor.matmul(
                        aA[:, i0:i0 + iw], kb[tc4][ro:ro + 32, 0:128],
                        qb[tc4][ro:ro + 32, i0:i0 + iw], start=True, stop=True,
                        tile_position=(ro, 0))
                for i0, iw in ISL:
                    nc.tensor.matmul(
                        aB[0:68, i0:i0 + iw], kb[tc4][ro:ro + 32, 128:NK],
                        qb[tc4][ro:ro + 32, i0:i0 + iw], start=True, stop=True,
                        tile_position=(ro, 0))
                p = wk.tile([128, 2 * N], BF16, tag=f"pa{h}", bufs=1)
                if s == 0:
                    nc.vector.memset(p[64:128, N:2 * N], 0.0)
                nc.scalar.activation(out=p[:, 0:N], in_=aA[:, 0:N], func=AF.Exp)
                nc.scalar.activation(out=p[0:68, N:2 * N], in_=aB[0:68, 0:N],
                                     func=AF.Exp)
                nc.vector.tensor_mul(out=p, in0=p, in1=ECt(h))
                pa[h] = p

            rS = [None, None]

            def f2(tc4):
                Sp = pat("Sps")
                for i0, iw in ISL:
                    for qq in range(4):
                        h = tc4 * 4 + qq
                        nc.tensor.matmul(
                            Sp[:, i0:i0 + iw], BH(qq)[0:128, :],
                            pa[h][:, i0:i0 + iw], start=(qq == 0), stop=False)
                        nc.tensor.matmul(
                            Sp[:, i0:i0 + iw], BH(qq)[0:68, :],
                            pa[h][0:68, N + i0:N + i0 + iw], start=False,
                            stop=(qq == 3))
                r = wk.tile([128, N], F32, tag=f"rS{tc4}", bufs=1)
                nc.vector.reciprocal_approx_fast(out=r, in_=Sp[:, 0:N])
                rS[tc4] = r

            tnb = [None, None]

            def f3(tc4):
                tun = pat("tun")
                for qq in range(4):
                    h = tc4 * 4 + qq
                    ro = 32 * qq
                    for i0, iw in ISL:
                        nc.tensor.matmul(
                            tun[ro:ro + 32, i0:i0 + iw],
                            vb[0][0:128, 32 * h:32 * h + 32],
                            pa[h][:, i0:i0 + iw], start=True, stop=False,
                            tile_position=(0, ro))
                        nc.tensor.matmul(
                            tun[ro:ro + 32, i0:i0 + iw],
                            vb[1][0:68, 32 * h:32 * h + 32],
                            pa[h][0:68, N + i0:N + i0 + iw], start=False,
                            stop=True, tile_position=(0, ro))
                t = wk.tile([128, N], BF16, tag=f"tnb{tc4}", bufs=1)
                nc.vector.tensor_mul(out=t, in0=tun[:, 0:N], in1=rS[tc4])
                tnb[tc4] = t

            for h in range(4):
                f1_head(h)
            f2(0)
            for h in range(4, 8):
                f1_head(h)
            f2(1)
            f3(0)
            f3(1)
            st["tnb"] = tnb

        def f4(s, st):
            tnb = st["tnb"]
            x2 = []
            for ch in range(2):
                t = wk.tile([128, N], F32, tag=f"x2{ch}", name=f"x2{ch}")
                x2.append(t)
            for j in range(8):
                n0 = j * 98
                po = psm("oproj")
                nc.tensor.matmul(
                    po[0:98, 0:C], ones1[0:1, 0:98], BO_R, start=True,
                    stop=False)
                for tc4 in range(2):
                    nc.tensor.matmul(
                        po[0:98, 0:C], tnb[tc4][:, n0:n0 + 98], WOT(tc4),
                        start=False, stop=(tc4 == 1))
                osb = wk.tile([128, C], F32, tag="osb", bufs=3)
                nc.vector.tensor_copy(out=osb[0:98, :], in_=po[0:98, 0:C])
                # raw reinterpret [98,256] -> rows 32j:32j+32 of [256, 784]
                # via a flat DRAM bounce (98*256 == 32*784)
                nc.sync.dma_start(
                    out=scr_d[s, n0 * C:(n0 + 98) * C].rearrange(
                        "(n c) -> n c", c=C),
                    in_=osb[0:98, :])
                nc.scalar.dma_start(
                    out=x2[j // 4][32 * (j % 4):32 * (j % 4) + 32, :],
                    in_=scr_d[s, j * 25088:(j + 1) * 25088].rearrange(
                        "(a i) -> a i", i=N))
            st["x2"] = x2

        def ln2_ffn(s, st):
            x1, x2 = st["x1"], st["x2"]
            st6 = wk.tile([128, 8], F32, tag="st6b")
            for ch in range(2):
                nc.vector.scalar_tensor_tensor(
                    out=x2[ch], in0=x2[ch], scalar=0.0, in1=x1[ch],
                    op0=ALU.add, op1=ALU.add,
                    accum_out=st6[:, ch:ch + 1])
            nc.vector.memset(st6[:, 2:4], 0.0)
            scr = wk.tile([128, N], BF16, tag="lnsc")
            for ch in range(2):
                nc.vector.scalar_tensor_tensor(
                    out=scr, in0=x2[ch], scalar=1.0, in1=x2[ch],
                    op0=ALU.mult, op1=ALU.mult,
                    accum_out=st6[:, 4 + ch:5 + ch])
            x2b = []
            for ch in range(2):
                t = wk.tile([128, N], BF16, tag=f"x2b{ch}", bufs=1)
                nc.vector.tensor_copy(out=t, in_=x2[ch])
                x2b.append(t)
            mean2, rst2 = ln_finish(st6, "l2")
            # fold LN2 affine into the c1-gelu scale/bias
            sc8 = wk.tile([128, 8], F32, tag="sc8")
            bc8 = wk.tile([128, 8], F32, tag="bc8")
            nc.vector.tensor_scalar(
                out=sc8, in0=A1cols, scalar1=rst2, scalar2=None, op0=ALU.mult)
            nc.vector.tensor_scalar(
                out=bc8, in0=RSC1, scalar1=mean2, scalar2=None, op0=ALU.mult)
            nc.vector.tensor_mul(out=bc8, in0=bc8, in1=sc8)
            nc.vector.tensor_sub(out=bc8, in0=B1cols, in1=bc8)
            # fold BN3 bias into x2 (after stats + cast consumed it)
            for ch in range(2):
                nc.vector.tensor_scalar(
                    out=x2[ch], in0=x2[ch], scalar1=B3cols[:, ch:ch + 1],
                    scalar2=None, op0=ALU.add)
            h1p = []
            for mc in range(8):
                p1 = pat("c1")
                for kc in range(2):
                    for i0, iw in ISL:
                        nc.tensor.matmul(
                            p1[:, i0:i0 + iw],
                            C1T(kc)[:, mc * 128:(mc + 1) * 128],
                            x2b[kc][:, i0:i0 + iw],
                            start=(kc == 0), stop=(kc == 1))
                hp = wk.tile([128, 30, 30], BF16, tag=f"h1p{mc}", bufs=1)
                if s == 0:
                    nc.vector.memset(hp, 0.0)
                nc.scalar.activation(
                    out=hp[:, 1:29, 1:29],
                    in_=p1[:, 0:N].rearrange("p (h w) -> p h w", w=W),
                    func=AF.Gelu, scale=sc8[:, mc:mc + 1],
                    bias=bc8[:, mc:mc + 1])
                h1p.append(hp)
            h2 = []
            for mc in range(8):
                pd = pat("dw2")
                for t9 in range(9):
                    dy, dx = t9 // 3, t9 % 3
                    nc.tensor.matmul(
                        pd[:, 0:392], DGDW2(mc, t9),
                        h1p[mc][:, dy:dy + 14, dx:dx + 28],
                        start=(t9 == 0), stop=(t9 == 8))
                    nc.tensor.matmul(
                        pd[:, 512:904], DGDW2(mc, t9),
                        h1p[mc][:, dy + 14:dy + 28, dx:dx + 28],
                        start=(t9 == 0), stop=(t9 == 8))
                t = wk.tile([128, N], BF16, tag=f"h2{mc}", bufs=1)
                nc.scalar.activation(
                    out=t.rearrange("p (b x) -> p b x", x=392),
                    in_=pd.rearrange("p (b x) -> p b x", x=512)[:, :, 0:392],
                    func=AF.Gelu, scale=A2c(mc), bias=B2c(mc))
                h2.append(t)
            for mc in range(2):
                p2 = pat("c2")
                for kc in range(8):
                    for i0, iw in ISL:
                        nc.tensor.matmul(
                            p2[:, i0:i0 + iw],
                            C2T(kc)[:, mc * 128:(mc + 1) * 128],
                            h2[kc][:, i0:i0 + iw],
                            start=(kc == 0), stop=(kc == 7))
                t3 = wk.tile([128, N], F32, tag="t3")
                nc.vector.scalar_tensor_tensor(
                    out=t3, in0=p2[:, 0:N], scalar=A3c(mc), in1=x2[mc],
                    op0=ALU.mult, op1=ALU.add)
                nc.sync.dma_start(
                    out=yv[s, mc * 128:(mc + 1) * 128, :], in_=t3)

        # ---------------- pipeline ----------------
        states = [None] * S
        states[0] = front(0, x0)
        for s in range(S):
            attn(s, states[s])
            f4(s, states[s])
            if s + 1 < S:
                states[s + 1] = front(s + 1, None)
            ln2_ffn(s, states[s])

    nc.finalize()
    _CACHE["nc"] = nc
    return nc


def _prep(inputs):
    if "shared" in _CACHE:
        return _CACHE["shared"]
    bf16 = ml_dtypes.bfloat16
    f32 = np.float32
    ii = {k: np.asarray(v, dtype=f32) for k, v in inputs.items() if k != "x"}

    rng = np.arange(128)

    def put_diag(arr, off, w):
        arr[rng, off + rng] = w

    dgw = np.zeros((128, DGW_COLS), f32)
    lpu_w = ii["lpu_w"].reshape(C, 9)
    for g in range(2):
        dgw[:, g * 9:(g + 1) * 9] = lpu_w[g * 128:(g + 1) * 128, :]
    dw_w = ii["dw_w"].reshape(C, 4)
    for g in range(2):
        dgw[:, 18 + g * 4:18 + (g + 1) * 4] = dw_w[g * 128:(g + 1) * 128, :]
    dw2_w = ii["dw2_w"].reshape(CM, 9)
    for m in range(8):
        dgw[:, 26 + m * 9:26 + (m + 1) * 9] = dw2_w[m * 128:(m + 1) * 128, :]

    ceb = np.zeros((128, CEB_COLS), f32)
    for name, off in (("wq", O_WQT), ("wk", O_WKT), ("wv", O_WVT)):
        w = ii[name]
        for kc in range(2):
            ceb[:, off + kc * 256:off + (kc + 1) * 256] = \
                w[:, kc * 128:(kc + 1) * 128].T
    for q in range(4):
        ceb[:, O_BH + q * 128 + 32 * q:O_BH + q * 128 + 32 * q + 32] = 1.0
    ceb[0, O_ROWB:O_ROWB + 256] = ii["bo"]
    ceb[0, O_ROWB + 256:O_ROWB + 512] = ii["bv"]

    cm = np.zeros((128, CMID_COLS), f32)
    pe = np.exp(ii["pos_b"][0])  # [8, 784, 196]
    for h in range(HEADS):
        et = pe[h].T  # [196, 784]
        cm[0:128, O_EC + h * 1568:O_EC + h * 1568 + 784] = et[0:128]
        cm[0:68, O_EC + h * 1568 + 784:O_EC + (h + 1) * 1568] = et[128:196]
    wo = ii["wo"]
    for kc in range(2):
        cm[:, O_WOT + kc * 256:O_WOT + (kc + 1) * 256] = \
            wo[:, kc * 128:(kc + 1) * 128].T

    cl = np.zeros((128, CL_COLS), f32)
    c1w = ii["c1_w"].reshape(CM, C)
    for kc in range(2):
        cl[:, O_C1T + kc * 1024:O_C1T + (kc + 1) * 1024] = \
            c1w[:, kc * 128:(kc + 1) * 128].T
    c2w = ii["c2_w"].reshape(C, CM)
    for kc in range(8):
        cl[:, O_C2T + kc * 256:O_C2T + (kc + 1) * 256] = \
            c2w[:, kc * 128:(kc + 1) * 128].T

    cbias = np.zeros((128, 10), f32)
    cbias[:, 0:2] = ii["lpu_b"].reshape(2, 128).T
    cbias[:, 2:4] = ii["dw_b"].reshape(2, 128).T
    cbias[:, 4:6] = ii["bk"].reshape(2, 128).T
    cbias[:, 6:8] = (-SCALE * ii["wq"].sum(axis=1)).reshape(2, 128).T
    cbias[:, 8:10] = (SCALE * ii["bq"]).reshape(2, 128).T

    def bnfold(g, b, m, v, cb, ngrp):
        A = g / np.sqrt(v + EPS)
        B = b - m * A + A * cb
        return (A.reshape(ngrp, 128).T.astype(f32),
                B.reshape(ngrp, 128).T.astype(f32))

    A1, B1 = bnfold(ii["bn1_g"], ii["bn1_b"], ii["bn1_m"], ii["bn1_v"],
                    ii["c1_b"], 8)
    A2, B2 = bnfold(ii["bn2_g"], ii["bn2_b"], ii["bn2_m"], ii["bn2_v"],
                    ii["dw2_b"], 8)
    A3, B3 = bnfold(ii["bn3_g"], ii["bn3_b"], ii["bn3_m"], ii["bn3_v"],
                    ii["c2_b"], 2)
    rsc1 = ii["c1_w"].reshape(CM, C).sum(axis=1).reshape(8, 128).T
    bncol = np.concatenate([A1, B1, A2, B2, A3, B3, rsc1], axis=1)

    shared = {
        "dgw": np.ascontiguousarray(dgw),
        "ceb": np.ascontiguousarray(ceb.astype(bf16)),
        "cmid": np.ascontiguousarray(cm.astype(bf16)),
        "clate": np.ascontiguousarray(cl.astype(bf16)),
        "cbias": np.ascontiguousarray(cbias),
        "bncol": np.ascontiguousarray(bncol.astype(f32)),
    }
    _CACHE["shared"] = shared
    return shared


def kernel(**inputs):
    nc = _build()
    x = np.ascontiguousarray(inputs["x"], dtype=np.float32)
    shared = _prep(inputs)
    in_maps = []
    for c in range(NCORES):
        m = dict(shared)
        m["x"] = np.ascontiguousarray(x[c * S:(c + 1) * S])
        in_maps.append(m)
    res = run_bass_kernel_spmd(nc, in_maps, core_ids=list(range(NCORES)))
    out = np.concatenate([res.results[c]["y"] for c in range(NCORES)], axis=0)
    return out
